# revision 54
# baseline (speedup 1.0000x reference)
"""Trainium2 Bass kernel for nn_Local2FWLRefine (gnn message passing).

Strategy
--------
The reference computes, per wedge w = (edge i->k, edge k->j) with (i,j) in E2:
    z[w]   = rho_in[w] @ w1 + b1          (rho_in 865 wide)
    msg[w] = silu(z[w]) @ w2 + b2
    M      = segment_sum(msg, eij)        ([E2, 128])
    out    = t_e2 + sigmoid(M@wgw+bgw) * tanh(t_e2@wgt+bgt)

The 865-wide matmul decomposes into per-edge projections:
    z[w] = Q1[eik[w]] + Q2[ekj[w]] + Q3[eij[w]] + c[w] * w1[864]
where Q1/Q2 are per-e1-edge tables and Q3 is per-e2-edge (b1 folded in),
and segment_sum(silu(z) @ w2) = segment_sum(silu(z)) @ w2.

Sharding: wedges sorted by eij; E2 split into 512-edge groups, groups
assigned contiguously to the 8 cores, so each core owns a disjoint slice
of the output rows (no all-reduce).  Phase 1 computes the Q tables on
device from host-staged (compacted, pre-transposed) feature blocks;
phase 2 gathers 3x128 floats per wedge with dma_gather, runs the MLP,
and accumulates the segment sum as silu_z^T @ S (S a 0/1 matrix built
with iota + is_equal) into PSUM, then applies the gated tail in
transposed orientation.
"""

import math
import os
import sys

sys.path.insert(0, "/opt/trn_rl_repo")

import ml_dtypes
import numpy as np

import concourse.bass as bass
import concourse.mybir as mybir
import concourse.tile as tile
from concourse import bacc
from concourse.bass_utils import run_bass_kernel_spmd
from concourse.tile import add_dep_helper
from concourse.masks import make_identity

P = 128
HID = 128
NRBF = 32
GRP = 512           # e2 edges per group (one PSUM bank of fp32)
NCORES = 8
F32 = mybir.dt.float32
F32R = mybir.dt.float32r
I16 = mybir.dt.int16


# ---------------------------------------------------------------- host index math
def _wedge_indices(edge_index1, edge_index2, num_nodes):
    src1 = np.asarray(edge_index1[0])
    dst1 = np.asarray(edge_index1[1])
    src2 = np.asarray(edge_index2[0])
    dst2 = np.asarray(edge_index2[1])
    nz = src1 != dst1
    s, d = src1[nz], dst1[nz]
    eid = np.nonzero(nz)[0]
    out_deg = np.bincount(s, minlength=num_nodes)
    out_order = np.argsort(s, kind="stable")
    out_ptr = np.concatenate([np.zeros(1, np.int64), np.cumsum(out_deg)])
    reps = out_deg[d]
    total = int(reps.sum())
    if total == 0:
        z = np.zeros(0, np.int64)
        return z, z, z, z, z, z
    starts = np.cumsum(reps) - reps
    local = np.arange(total) - np.repeat(starts, reps)
    kj_f = out_order[np.repeat(out_ptr[d], reps) + local]
    i = np.repeat(s, reps)
    k = np.repeat(d, reps)
    eik = np.repeat(eid, reps)
    j = d[kj_f]
    ekj = eid[kj_f]
    m = i != j
    i, k, j, eik, ekj = i[m], k[m], j[m], eik[m], ekj[m]
    e2_keys = src2.astype(np.int64) * num_nodes + dst2
    pk = i.astype(np.int64) * num_nodes + j
    pos = np.searchsorted(e2_keys, pk)
    posc = np.minimum(pos, e2_keys.size - 1)
    valid = (pos < e2_keys.size) & (e2_keys[posc] == pk)
    return i[valid], k[valid], j[valid], eik[valid], ekj[valid], posc[valid]


def _wrap16(arr):
    """int16 index array -> [128, n/16] layout dma_gather expects
    (index i at partition i%16, col i//16; replicated to all 8 Q7 cores)."""
    a = arr.astype(np.int16).reshape(-1, 16).T
    return np.ascontiguousarray(np.tile(a, (8, 1)))


def host_prep(t_e2, h, edge_index1, edge_index2, e1_to_e2, rbf_e1, rbf_e2,
              sph_e1, num_nodes, w1, b1, w2, b2, wgw, bgw, wgt, bgt):
    E2 = t_e2.shape[0]
    N = int(num_nodes)
    E1 = rbf_e1.shape[0]
    src1 = np.asarray(edge_index1[0]).astype(np.int64)
    dst1 = np.asarray(edge_index1[1]).astype(np.int64)
    e1e2 = np.asarray(e1_to_e2).astype(np.int64)

    i_, k_, j_, eik, ekj, eij = _wedge_indices(edge_index1, edge_index2, N)
    W0 = eik.size
    if W0 == 0:
        return None  # caller returns t_e2 unchanged

    c_w = (np.asarray(sph_e1)[eik, 1] * np.asarray(sph_e1)[ekj, 1]).astype(np.float32)
    order = np.argsort(eij, kind="stable")
    eik, ekj, eij, c_w = eik[order], ekj[order], eij[order], c_w[order]

    NGT = math.ceil(E2 / GRP)
    NG = math.ceil(NGT / NCORES)
    NGE = NG * GRP

    gix = eij // GRP                      # global group slot of each wedge (sorted)
    nslots = NCORES * NG
    counts = np.bincount(gix, minlength=nslots)
    SUBG = max(1, int(math.ceil(counts.max() / P)))
    GW = SUBG * P
    WP = NG * GW
    NBLK = WP // P

    # group slot boundaries in the sorted wedge arrays
    bnd = np.searchsorted(gix, np.arange(nslots + 1))

    cnt_full = np.bincount(eij, minlength=E2).astype(np.float32)

    E1 = src1.size
    cores = []
    U12s = []
    rots = []
    for c in range(NCORES):
        base_e = c * NGE
        w_lo, w_hi = bnd[c * NG], bnd[(c + 1) * NG]
        ceik, cekj, ceij, ccw = (eik[w_lo:w_hi], ekj[w_lo:w_hi],
                                 eij[w_lo:w_hi], c_w[w_lo:w_hi])
        U12 = np.unique(np.concatenate([ceik, cekj])) if ceik.size else \
            np.zeros(1, np.int64)
        # order the per-core edge table by ring-relative id so a core whose
        # wedges straddle the wrap still sees contiguous table positions
        rot = int(ceik.min()) if ceik.size else 0
        U12 = U12[np.argsort((U12 - rot) % E1, kind="stable")]
        rots.append(rot)
        U12s.append(U12)
        cores.append((base_e, w_lo, w_hi, ceik, cekj, ceij, ccw, U12))

    # ---- per-group table segments ----
    # Each core's q1/q2 table is laid out in NG segments with SHARED bases
    # B_g: segment g holds the core's U12 rows [c_g, hi_g] (c_g = first row
    # any group-g wedge reads as Q1; hi_g covers the overlap into the next
    # segment), so per-block Q1 row windows line up across cores.  Rows used
    # by several groups are duplicated into each segment.
    seg_lo = np.zeros((NCORES, NG), np.int64)   # c_g per core
    seg_hi = np.zeros((NCORES, NG), np.int64)   # hi_g per core (inclusive)

    def u12_pos(c, ids):
        """positions of edge ids in the (ring-rotated) per-core table"""
        U12, rot = U12s[c], rots[c]
        keys = (U12 - rot) % E1
        return np.searchsorted(keys, (ids - rot) % E1)

    for c, (base_e, w_lo, w_hi, ceik, cekj, ceij, ccw, U12) in enumerate(cores):
        p1 = u12_pos(c, ceik)
        p2 = u12_pos(c, cekj)
        for g in range(NG):
            lo = bnd[c * NG + g] - w_lo
            hi = bnd[c * NG + g + 1] - w_lo
            if hi > lo:
                seg_lo[c, g] = int(p1[lo:hi].min())
                seg_hi[c, g] = int(max(p1[lo:hi].max(), p2[lo:hi].max()))
            else:
                seg_lo[c, g] = seg_lo[c, g - 1] if g else 0
                seg_hi[c, g] = seg_lo[c, g]
    # segment g must cover all rows in [c_g, c_{g+1}) plus the Q1/Q2 overlap
    nxt = np.concatenate([seg_lo[:, 1:],
                          np.array([[u.size for u in U12s]]).T], axis=1)
    seg_len = np.maximum(seg_hi + 1, nxt) - seg_lo        # [NCORES, NG]
    L = seg_len.max(axis=0)                               # shared lengths
    B = np.zeros(NG + 1, np.int64)
    for g in range(NG):
        B[g + 1] = B[g] + ((int(L[g]) + 63) // 64) * 64
    T = max(512, int(math.ceil(B[NG] / (4 * P))) * 4 * P)
    if T >= 32768:
        raise RuntimeError(f"per-core Q table too large for int16 gather: {T}")
    NB1 = T // P
    NB2 = NGE // P

    def u12_to_slot(c):
        """[NG, U12.size] -> slot id (or -1) per (segment, row)."""
        n = U12s[c].size
        slot_of = np.full((NG, n), -1, np.int64)
        for g in range(NG):
            lo, ln = seg_lo[c, g], seg_len[c, g]
            slot_of[g, lo:lo + ln] = B[g] + np.arange(ln)
        return slot_of

    # padded per-(core,group,subtile) el / q1-row values to derive shared
    # window bases
    el_pad = np.full((NCORES, NG, SUBG, P), np.nan, np.float32)
    r1_pad = np.full((NCORES, NG, SUBG, P), np.nan, np.float32)
    percore = []
    for c, (base_e, w_lo, w_hi, ceik, cekj, ceij, ccw, U12) in enumerate(cores):
        q2i = np.zeros(WP, np.int16)
        cwp = np.zeros(WP, np.float32)
        elg = np.full(WP, np.nan, np.float32)   # el within group [0, GRP)
        r1g = np.full(WP, np.nan, np.float32)   # q1 table slot
        p1 = u12_pos(c, ceik)
        p2 = u12_pos(c, cekj)
        slot_of = u12_to_slot(c)
        loc = ceij - base_e
        hi12c = np.zeros(NG, np.int64)
        for g in range(NG):
            lo = bnd[c * NG + g] - w_lo
            hi = bnd[c * NG + g + 1] - w_lo
            n = hi - lo
            dst = g * GW
            s1 = slot_of[g, p1[lo:hi]]
            s2 = slot_of[g, p2[lo:hi]]
            assert n == 0 or (s1.min() >= 0 and s2.min() >= 0)
            q2i[dst:dst + n] = s2
            cwp[dst:dst + n] = ccw[lo:hi]
            elg[dst:dst + n] = (loc[lo:hi] - g * GRP).astype(np.float32)
            r1g[dst:dst + n] = s1.astype(np.float32)
            hi12c[g] = s2.max(initial=0)
        el_pad[c] = elg.reshape(NG, SUBG, P)
        r1_pad[c] = r1g.reshape(NG, SUBG, P)
        percore.append((q2i, cwp, hi12c))

    # shared (across cores) per-(g,s) window base; WS = max span, mult of 32
    with np.errstate(invalid="ignore"):
        mn = np.nanmin(el_pad, axis=(0, 3))     # [NG, SUBG]
        mx = np.nanmax(el_pad, axis=(0, 3))
    valid_any = ~np.isnan(el_pad).all(axis=(0, 3))        # [NG, SUBG]
    mn = np.where(np.isnan(mn), 0.0, mn)
    mx = np.where(np.isnan(mx), 0.0, mx)
    span = (mx - mn + 1).max()
    WS = min(GRP, int(math.ceil(span / 32)) * 32)
    base_gs = np.minimum(mn, GRP - WS).astype(np.int32)   # [NG, SUBG]

    # Q3 64-aligned windows per block: els of block (g,s) fit in
    # [wb, wb+128) with wb = 64*floor(mn/64) (cross-core span <= WS <= 64,
    # enforced below).  Window A = [wb, wb+64), window B = [wb+64, wb+128)
    # (only when some el >= wb+64).  The one-hot rows live at el%128 so the
    # lhsT slice's base partition always matches the rhs table slice's.
    if WS > 64:
        raise RuntimeError(f"q3 window scheme needs span <= 64, got WS={WS}")
    wb_gs = (64 * (mn.astype(np.int64) // 64)).astype(np.int32)   # [NG, SUBG]
    useB_gs = (mx >= wb_gs + 64) & valid_any
    useA_gs = valid_any

    # same 64-aligned window scheme for the Q1 table rows
    with np.errstate(invalid="ignore"):
        mn1 = np.nanmin(r1_pad, axis=(0, 3))
        mx1 = np.nanmax(r1_pad, axis=(0, 3))
    mn1 = np.where(np.isnan(mn1), 0.0, mn1)
    mx1 = np.where(np.isnan(mx1), 0.0, mx1)
    if (mx1 - mn1 + 1).max() > 64:
        raise RuntimeError(
            f"q1 window scheme needs span <= 64, got {(mx1 - mn1 + 1).max()}")
    wb1_gs = (64 * (mn1.astype(np.int64) // 64)).astype(np.int32)
    useB1_gs = (mx1 >= wb1_gs + 64) & valid_any
    useA1_gs = valid_any

    # per-group q12 block-batch milestone (in 4*P-row units, shared over cores)
    hi12 = np.max(np.stack([pc[2] for pc in percore]), axis=0)
    need_b4 = tuple(int(x) for x in np.minimum(hi12 // (4 * P) + 1, NB1 // 4))

    meta = dict(NG=NG, SUBG=SUBG, T=T, NB1=NB1, NB2=NB2, NGE=NGE, WP=WP,
                NBLK=NBLK, WS=WS, bases=tuple(map(int, base_gs.reshape(-1))),
                need_b4=need_b4,
                q3wb=tuple(map(int, wb_gs.reshape(-1))),
                q3useA=tuple(map(bool, useA_gs.reshape(-1))),
                q3useB=tuple(map(bool, useB_gs.reshape(-1))),
                q1wb=tuple(map(int, wb1_gs.reshape(-1))),
                q1useA=tuple(map(bool, useA1_gs.reshape(-1))),
                q1useB=tuple(map(bool, useB1_gs.reshape(-1))))

    # ---- weights (shared) ----
    w1 = np.asarray(w1, np.float32)
    wcat = np.zeros((4 * P, 2 * P), np.float32)
    wcat[0:128, 0:128] = w1[0:128]          # t_e2[e1e2[e]]  -> Q1
    wcat[0:128, 128:256] = w1[128:256]      # t_e2[e1e2[e]]  -> Q2
    wcat[128:160, 0:128] = w1[768:800]      # rbf_e1[e]      -> Q1
    wcat[128:160, 128:256] = w1[800:832]    # rbf_e1[e]      -> Q2
    wcat[160:288, 0:128] = w1[384:512]      # h[src1[e]]     -> Q1
    wcat[288:416, 0:128] = w1[512:640]      # h[dst1[e]]     -> Q1 (h_k)
    wcat[288:416, 128:256] = w1[640:768]    # h[dst1[e]]     -> Q2 (h_j)
    wcat[416, 0:128] = np.asarray(b1, np.float32)   # b1 via const column
    # gate sigmoid via tanh identity: sigmoid(x) = 0.5*(1 + tanh(x/2)); the
    # 1/2 is folded into wgw/bgw, and M = U@w2 + cnt x b2 is never
    # materialized: M@(wgw/2) = U@(w2@wgw/2) + cnt x (b2@wgw/2).
    wgwh = np.asarray(wgw, np.float32) * 0.5
    bgwh = np.asarray(bgw, np.float32) * 0.5
    w2w = (np.asarray(w2, np.float32) @ wgwh).astype(np.float32)
    b2w = (np.asarray(b2, np.float32) @ wgwh).astype(np.float32)
    shared = {
        "wcat": np.ascontiguousarray(wcat).astype(ml_dtypes.bfloat16),
        "w1c": np.ascontiguousarray(w1[256:384]).astype(ml_dtypes.bfloat16),
        "w1f": np.ascontiguousarray(w1[832:864]).astype(ml_dtypes.bfloat16),
        "w2w": w2w,
        "b2w": b2w[None, :],
        "wgt": np.asarray(wgt, np.float32).astype(ml_dtypes.bfloat16),
        "bgwc": np.ascontiguousarray(bgwh[:, None]),
        "bgtc": np.ascontiguousarray(np.asarray(bgt, np.float32)[:, None]),
        "w1lr": np.ascontiguousarray(w1[864:865, :]).astype(ml_dtypes.bfloat16),
    }

    t_e2 = np.asarray(t_e2, np.float32)
    h = np.asarray(h, np.float32)
    rbf_e1 = np.asarray(rbf_e1, np.float32)
    rbf_e2 = np.asarray(rbf_e2, np.float32)

    el_rel = el_pad.reshape(NCORES, NG, SUBG, P) - base_gs[None, :, :, None]
    el_rel = np.where(np.isnan(el_rel), -5.0, el_rel).astype(np.float32)

    in_maps = []
    for c, (base_e, w_lo, w_hi, ceik, cekj, ceij, ccw, U12) in enumerate(cores):
        q2i, cwp, _ = percore[c]
        # slot t -> U12 row (duplicated across segment overlaps)
        slotrow = np.full(T, -1, np.int64)
        for g in range(NG):
            ln = int(seg_len[c, g])
            slotrow[B[g]:B[g] + ln] = seg_lo[c, g] + np.arange(ln)
        mask = slotrow >= 0
        U12e = U12[np.clip(slotrow, 0, U12.size - 1)]
        gtab = np.zeros((T, 4 * P), np.float32)
        gtab[:, 416] = 1.0          # constant column carrying b1
        gtab[mask, 0:128] = t_e2[e1e2[U12e[mask]]]
        gtab[mask, 128:160] = rbf_e1[U12e[mask]]
        gtab[mask, 160:288] = h[src1[U12e[mask]]]
        gtab[mask, 288:416] = h[dst1[U12e[mask]]]
        gtabT = np.ascontiguousarray(
            gtab.reshape(NB1, P, 4 * P).transpose(0, 2, 1)).astype(
                ml_dtypes.bfloat16)

        hi_e = min(base_e + NGE, E2)
        nreal = hi_e - base_e
        tslab = np.zeros((NGE, P), np.float32)
        rbf2s = np.zeros((NGE, NRBF), np.float32)
        cntc = np.zeros(NGE, np.float32)
        if nreal > 0:
            tslab[:nreal] = t_e2[base_e:hi_e]
            rbf2s[:nreal] = rbf_e2[base_e:hi_e]
            cntc[:nreal] = cnt_full[base_e:hi_e]

        # one-hot expansion matrices, interleaved per subtile:
        #   sel13[g, el%128, s, 0, p] = 1   (Q3: group-local edge index el)
        #   sel13[g, row%128, s, 1, p] = 1  (Q1: q1-table row)
        sel13 = np.zeros((NG, P, SUBG, 2, P), ml_dtypes.bfloat16)
        elc = el_pad[c]                     # [NG, SUBG, P], el within group
        gg, ss, pp = np.nonzero(~np.isnan(elc))
        rr = elc[gg, ss, pp].astype(np.int64) % P
        sel13[gg, rr, ss, 0, pp] = 1.0
        r1c = r1_pad[c]
        gg, ss, pp = np.nonzero(~np.isnan(r1c))
        rr = r1c[gg, ss, pp].astype(np.int64) % P
        sel13[gg, rr, ss, 1, pp] = 1.0

        in_maps.append({
            "gtabT": gtabT,
            "t16": np.ascontiguousarray(tslab.T).astype(ml_dtypes.bfloat16),
            "rbf16": np.ascontiguousarray(rbf2s.T).astype(ml_dtypes.bfloat16),
            "cnt": np.ascontiguousarray(cntc[None, :]),
            "q2i": _wrap16(q2i),
            "sel13": np.ascontiguousarray(sel13).reshape(NG, P, SUBG, 2 * P),
            "cwt": np.ascontiguousarray(cwp[None, :]).astype(
                ml_dtypes.bfloat16),
            "elw": np.ascontiguousarray(
                el_rel[c].reshape(NBLK, P).T),
            **shared,
        })
    return in_maps, meta, E2


# ---------------------------------------------------------------- device program
def build_program(meta, use_silu=True):
    NG, SUBG, T = meta["NG"], meta["SUBG"], meta["T"]
    NB1, NB2, NGE = meta["NB1"], meta["NB2"], meta["NGE"]
    WP, NBLK, WS = meta["WP"], meta["NBLK"], meta["WS"]
    bases = meta["bases"]
    GW = SUBG * P
    AF = mybir.ActivationFunctionType

    nc = bacc.Bacc("TRN2", target_bir_lowering=False, debug=False,
                   enable_asserts=False, num_devices=NCORES)

    def din(name, shape, dt=F32):
        return nc.dram_tensor(name, shape, dt, kind="ExternalInput").ap()

    BF16 = mybir.dt.bfloat16
    gtabT = din("gtabT", [NB1, 4 * P, P], BF16)
    t16d = din("t16", [P, NGE], BF16)
    rbf16d = din("rbf16", [NRBF, NGE], BF16)
    cnt = din("cnt", [1, NGE], F32R)
    q2i = din("q2i", [P, WP // 16], I16)
    sel13 = din("sel13", [NG, P, SUBG, 2 * P], BF16)
    cwt = din("cwt", [1, WP], BF16)
    elw = din("elw", [P, NBLK])
    wcat = din("wcat", [4 * P, 2 * P], BF16)
    w1c = din("w1c", [P, P], BF16)
    w1f = din("w1f", [NRBF, P], BF16)
    w2w = din("w2w", [P, P], F32R)
    b2w = din("b2w", [1, P], F32R)
    wgt = din("wgt", [P, P], BF16)
    bgwc = din("bgwc", [P, 1])
    bgtc = din("bgtc", [P, 1])
    w1lr = din("w1lr", [1, P], BF16)
    outT = nc.dram_tensor("outT", [P, NGE], BF16, kind="ExternalOutput").ap()

    with tile.TileContext(nc) as tc:
        with (
            tc.tile_pool(name="const", bufs=1) as cpool,
            tc.tile_pool(name="dram", bufs=1, space="DRAM") as dpool,
            tc.tile_pool(name="p1in", bufs=3) as p1in,
            tc.tile_pool(name="p1out", bufs=2) as p1out,
            tc.tile_pool(name="gath", bufs=3) as gath,
            tc.tile_pool(name="zbuf", bufs=3) as zbuf,
            tc.tile_pool(name="sbuf", bufs=3) as spool,
            tc.tile_pool(name="tail", bufs=2) as tpool,
            tc.tile_pool(name="ps1", bufs=2, space="PSUM") as ps1,
            tc.tile_pool(name="psu", bufs=2, space="PSUM") as psu,
            tc.tile_pool(name="psz", bufs=2, space="PSUM") as pszp,
            tc.tile_pool(name="pstail", bufs=2, space="PSUM") as pstail,
        ):
            # ---------------- constants ----------------
            wcat_sb = cpool.tile([P, 4, 2 * P], mybir.dt.bfloat16)
            nc.sync.dma_start(wcat_sb[:],
                              wcat.rearrange("(c p) f -> p c f", p=P))
            w1c_sb = cpool.tile([P, P], mybir.dt.bfloat16)
            nc.sync.dma_start(w1c_sb[:], w1c[:, :])
            w1f_sb = cpool.tile([NRBF, P], mybir.dt.bfloat16)
            nc.sync.dma_start(w1f_sb[:], w1f[:, :])
            w2w_sb = cpool.tile([P, P], F32R)
            nc.sync.dma_start(w2w_sb[:], w2w[:, :])
            b2w_sb = cpool.tile([1, P], F32R)
            nc.sync.dma_start(b2w_sb[:], b2w[:, :])
            wgt_sb = cpool.tile([P, P], mybir.dt.bfloat16)
            nc.sync.dma_start(wgt_sb[:], wgt[:, :])
            t16_sb = cpool.tile([P, NGE], mybir.dt.bfloat16)
            nc.sync.dma_start(t16_sb[:], t16d[:, :])
            bgw_sb = cpool.tile([P, 1], F32)
            nc.sync.dma_start(bgw_sb[:], bgwc[:, :])
            bgt_sb = cpool.tile([P, 1], F32)
            nc.sync.dma_start(bgt_sb[:], bgtc[:, :])
            w1lr_sb = cpool.tile([1, P], mybir.dt.bfloat16)
            nc.sync.dma_start(w1lr_sb[:], w1lr[:, :])
            cnt_sb = cpool.tile([1, NGE], F32R)
            nc.sync.dma_start(cnt_sb[:], cnt[:, :])

            elw_sb = cpool.tile([P, NBLK], F32)
            nc.sync.dma_start(elw_sb[:], elw[:, :])
            q2i_sb = cpool.tile([P, WP // 16], I16)
            nc.sync.dma_start(q2i_sb[:], q2i[:, :])
            zero_f = cpool.tile([1, GRP], F32)
            nc.gpsimd.memset(zero_f[:], 0.0)
            zero_sb = cpool.tile([1, GRP], F32R)
            nc.vector.tensor_copy(zero_sb[:], zero_f[:])
            ident_sb = cpool.tile([P, P], mybir.dt.bfloat16)
            make_identity(nc, ident_sb[:])
            iota_sb = cpool.tile([P, WS], F32)
            nc.gpsimd.iota(iota_sb[:], pattern=[[1, WS]], base=0,
                           channel_multiplier=0,
                           allow_small_or_imprecise_dtypes=True)

            # DRAM scratch table (q2 only; q1/q3 live in SBUF, row r at
            # partition r%128, free chunk r//128)
            q2t = dpool.tile([T, P], mybir.dt.bfloat16)
            q1sb = cpool.tile([P, NB1, P], mybir.dt.bfloat16)
            q3sb = cpool.tile([P, NB2, P], mybir.dt.bfloat16)

            # ---------------- phase 1: Q tables ----------------
            # q12 and q3 block batches are emitted interleaved, and each
            # batch's DRAM write is kept as a milestone so that phase-2
            # group g only waits for the table prefix it actually reads
            # (dma_gather's DRAM source read is not tracked by Tile's
            # dependency hook, hence the explicit deps).
            q12_ms = [None] * (NB1 // 4)
            q3_ms = [None] * (NB2 // 8)

            def emit_q12(b4i):
                q2c = p1out.tile([P, 4, P], mybir.dt.bfloat16, tag="q2c")
                gt = p1in.tile([P, 4, 4, P], mybir.dt.bfloat16, tag="gt")
                nc.sync.dma_start(
                    gt[:], gtabT[b4i * 4:b4i * 4 + 4]
                    .rearrange("n (c p) f -> p n c f", p=P))
                for half in range(4):
                    pq = ps1.tile([P, 2 * P], F32, tag="pq")
                    for ci in range(4):
                        nc.tensor.matmul(
                            pq[:], lhsT=gt[:, half, ci, :],
                            rhs=wcat_sb[:, ci, :],
                            start=(ci == 0), stop=(ci == 3))
                    nc.vector.tensor_copy(q1sb[:, b4i * 4 + half, :],
                                          pq[:, 0:P])
                    nc.vector.tensor_copy(q2c[:, half, :], pq[:, P:2 * P])
                q12_ms[b4i] = nc.scalar.dma_start(
                    q2t[b4i * 4 * P:(b4i + 1) * 4 * P, :]
                    .rearrange("(c p) f -> p c f", p=P),
                    q2c[:])

            def emit_q3(b8):
                rts8 = p1in.tile([NRBF, 8 * P], mybir.dt.bfloat16, tag="rts")
                nc.sync.dma_start(rts8[:],
                                  rbf16d[:, b8 * 8 * P:(b8 + 1) * 8 * P])
                for qi in range(8):
                    c0 = (b8 * 8 + qi) * P
                    pq3 = ps1.tile([P, P], F32, tag="pq")
                    nc.tensor.matmul(pq3[:], lhsT=t16_sb[:, c0:c0 + P],
                                     rhs=w1c_sb[:], start=True, stop=False)
                    nc.tensor.matmul(pq3[:], lhsT=rts8[:, qi * P:(qi + 1) * P],
                                     rhs=w1f_sb[:], start=False, stop=True)
                    q3_ms[b8] = nc.vector.tensor_copy(
                        q3sb[:, b8 * 8 + qi, :], pq3[:])

            for step in range(max(NB1 // 4, NB2 // 8)):
                if step < NB1 // 4:
                    emit_q12(step)
                if step < NB2 // 8:
                    emit_q3(step)

            # ---------------- phase 2: wedges + tail ----------------
            for g in range(NG):
                ic0 = g * GW // 16
                ic1 = (g + 1) * GW // 16
                g2 = gath.tile([P, SUBG, P], mybir.dt.bfloat16, tag="g2")
                gi2 = nc.gpsimd.dma_gather(
                    out_ap=g2[:], in_ap=q2t[:, :],
                    idxs_ap=q2i_sb[:, ic0:ic1],
                    num_idxs=GW, num_idxs_reg=GW, elem_size=P,
                    single_packet=False)
                selg = gath.tile([P, SUBG, 2, P], mybir.dt.bfloat16,
                                 tag="selg")
                nc.sync.dma_start(
                    selg[:],
                    sel13[g:g + 1].rearrange("n p s f -> p (n s) f")
                    .rearrange("p s (two f) -> p s two f", two=2))
                q12_need = q12_ms[meta["need_b4"][g] - 1]
                add_dep_helper(gi2.ins, q12_need.ins, sync=True,
                               reason="q2 prefix before gather")

                cwt_g = spool.tile([1, GW], mybir.dt.bfloat16, tag="cwt")
                nc.sync.dma_start(cwt_g[:], cwt[:, g * GW:(g + 1) * GW])
                pu = psu.tile([P, GRP], F32, tag="pu")
                nc.tensor.matmul(pu[:], lhsT=zero_sb[:, 0:P],
                                 rhs=zero_sb[:, 0:GRP],
                                 start=True, stop=False)

                quads = []
                q0 = 0
                while q0 < SUBG:
                    qw = min(4, SUBG - q0)
                    psz = pszp.tile([P, qw * P], F32, tag="psz")
                    # one PSUM accumulation group per psz tile: the first
                    # (start=True) matmul pending-zeroes the whole bank, the
                    # last rank-1 carries stop.  Matmuls are grouped by
                    # stationary operand to limit weight reloads.
                    for h0 in range(0, qw, 2):
                        hw_ = min(2, qw - h0)
                        dst = psz[:, h0 * P:(h0 + hw_) * P]
                        nc.tensor.matmul(dst, lhsT=ident_sb[:],
                                         rhs=g2[:, q0 + h0:q0 + h0 + hw_, :],
                                         start=(h0 == 0), stop=False)
                    for bi in range(qw):
                        s = q0 + bi
                        blk = g * SUBG + s
                        dst = psz[:, bi * P:(bi + 1) * P]
                        # one-hot window matmuls; when A and B sit in the
                        # same table rank (base partition 0) they merge into
                        # a single K=128 matmul
                        def sel_windows(base, useA, useB, table, sl):
                            if not useA:
                                return
                            p0 = base % P
                            rk = base // P
                            if useB and p0 == 0:
                                nc.tensor.matmul(
                                    dst, lhsT=selg[:, s, sl, :],
                                    rhs=table[:, rk, :],
                                    start=False, stop=False)
                                return
                            nc.tensor.matmul(
                                dst, lhsT=selg[p0:p0 + 64, s, sl, :],
                                rhs=table[p0:p0 + 64, rk, :],
                                start=False, stop=False)
                            if useB:
                                b0 = base + 64
                                q0_ = b0 % P
                                nc.tensor.matmul(
                                    dst, lhsT=selg[q0_:q0_ + 64, s, sl, :],
                                    rhs=table[q0_:q0_ + 64, b0 // P, :],
                                    start=False, stop=False)
                        sel_windows(g * GRP + meta["q3wb"][blk],
                                    meta["q3useA"][blk], meta["q3useB"][blk],
                                    q3sb, 0)
                        sel_windows(meta["q1wb"][blk],
                                    meta["q1useA"][blk], meta["q1useB"][blk],
                                    q1sb, 1)
                    for bi in range(qw):
                        sblk = q0 + bi
                        nc.tensor.matmul(
                            psz[:, bi * P:(bi + 1) * P],
                            lhsT=cwt_g[:, sblk * P:(sblk + 1) * P],
                            rhs=w1lr_sb[:],
                            start=False, stop=(bi == qw - 1))
                    silu = zbuf.tile([P, qw, P], mybir.dt.bfloat16, tag="silu")
                    if use_silu:
                        nc.scalar.activation(
                            silu[:].rearrange("p a b -> p (a b)"), psz[:],
                            AF.Silu)
                    else:
                        sig = zbuf.tile([P, qw, P], F32, tag="sig")
                        nc.scalar.activation(
                            sig[:].rearrange("p a b -> p (a b)"), psz[:],
                            AF.Sigmoid)
                        nc.vector.tensor_tensor(
                            out=silu[:].rearrange("p a b -> p (a b)"),
                            in0=sig[:].rearrange("p a b -> p (a b)"),
                            in1=psz[:], op=mybir.AluOpType.mult)
                    quads.append((q0, qw, silu))
                    q0 += qw

                for s in range(SUBG):
                    blk = g * SUBG + s
                    base = bases[g * SUBG + s]
                    ssb = spool.tile([P, WS], mybir.dt.bfloat16, tag="ssb")
                    nc.vector.tensor_scalar(
                        out=ssb[:], in0=iota_sb[:],
                        scalar1=elw_sb[:, blk:blk + 1], scalar2=None,
                        op0=mybir.AluOpType.is_equal)
                    qidx = s // 4
                    sq0, sqw, silu_q = quads[qidx]
                    nc.tensor.matmul(
                        pu[:, base:base + WS],
                        lhsT=silu_q[:, s - sq0, :], rhs=ssb[:],
                        start=False, stop=(s == SUBG - 1))

                # tail for this group's 512 edges:
                #   th = tanh(U@W2W + cnt x B2W + bgw/2)    (= 2*sigmoid-1)
                #   T  = tanh(t@wgt + bgt)
                #   out = t + 0.5*(1+th)*T
                u_sb = tpool.tile([P, GRP], F32R, tag="u")
                nc.scalar.activation(u_sb[:].bitcast(F32), pu[:],
                                     AF.Copy)
                pg = pstail.tile([P, GRP], F32, tag="ptail")
                for h0 in (0, 2 * P):
                    nc.tensor.matmul(pg[:, h0:h0 + 2 * P], lhsT=w2w_sb[:],
                                     rhs=u_sb[:, h0:h0 + 2 * P],
                                     start=True, stop=False)
                    nc.tensor.matmul(pg[:, h0:h0 + 2 * P], lhsT=b2w_sb[:],
                                     rhs=cnt_sb[:, g * GRP + h0:
                                                g * GRP + h0 + 2 * P],
                                     start=False, stop=True)
                th = tpool.tile([P, GRP], F32, tag="gate")
                nc.scalar.activation(th[:], pg[:], AF.Tanh, bias=bgw_sb[:])

                pt = pstail.tile([P, GRP], F32, tag="ptail")
                for h0 in (0, 2 * P):
                    nc.tensor.matmul(
                        pt[:, h0:h0 + 2 * P], lhsT=wgt_sb[:],
                        rhs=t16_sb[:, g * GRP + h0:g * GRP + h0 + 2 * P],
                        start=True, stop=True)
                tact = tpool.tile([P, GRP], F32, tag="tact")
                nc.scalar.activation(tact[:], pt[:], AF.Tanh, bias=bgt_sb[:])

                t32g = tpool.tile([P, GRP], F32, tag="t32")
                nc.scalar.activation(t32g[:],
                                     t16_sb[:, g * GRP:(g + 1) * GRP],
                                     AF.Copy)
                o_sb = tpool.tile([P, GRP], F32, tag="o")
                nc.vector.tensor_tensor(out=o_sb[:], in0=th[:], in1=tact[:],
                                        op=mybir.AluOpType.mult)
                nc.gpsimd.tensor_add(o_sb[:], o_sb[:], tact[:])
                nc.vector.tensor_scalar(
                    out=o_sb[:], in0=o_sb[:], scalar1=0.5, scalar2=None,
                    op0=mybir.AluOpType.mult)
                o16 = tpool.tile([P, GRP], mybir.dt.bfloat16, tag="o16")
                nc.vector.tensor_add(o16[:], o_sb[:], t32g[:])
                nc.scalar.dma_start(outT[:, g * GRP:(g + 1) * GRP], o16[:])

    nc.compile()
    return nc


_CACHE = {}


def _get_program(meta, use_silu=True):
    key = (tuple(sorted((k, v) for k, v in meta.items() if k != "bases")),
           meta["bases"], use_silu)
    if key not in _CACHE:
        _CACHE[key] = build_program(meta, use_silu=use_silu)
    return _CACHE[key]


def kernel(**inputs):
    np_inputs = {k: np.asarray(v) for k, v in inputs.items()}
    t_e2 = np.asarray(np_inputs["t_e2"], np.float32)
    prep = host_prep(
        t_e2, np_inputs["h"], np_inputs["edge_index1"],
        np_inputs["edge_index2"], np_inputs["e1_to_e2"], np_inputs["rbf_e1"],
        np_inputs["rbf_e2"], np_inputs["sph_e1"], np_inputs["num_nodes"],
        np_inputs["w1"], np_inputs["b1"], np_inputs["w2"], np_inputs["b2"],
        np_inputs["wgw"], np_inputs["bgw"], np_inputs["wgt"], np_inputs["bgt"])
    if prep is None:
        return t_e2
    in_maps, meta, E2 = prep
    use_silu = os.environ.get("KERNEL_NO_SILU", "0") != "1"
    nc = _get_program(meta, use_silu=use_silu)
    trace = os.environ.get("KERNEL_TRACE", "0") == "1"
    res = run_bass_kernel_spmd(nc, in_maps, core_ids=list(range(NCORES)),
                               trace=trace)
    kernel.last_results = res
    NGE = meta["NGE"]
    out = np.empty((E2, HID), np.float32)
    for c in range(NCORES):
        base = c * NGE
        hi = min(base + NGE, E2)
        if hi <= base:
            break
        out[base:hi, :] = res.results[c]["outT"][:, :hi - base].T
    return out


kernel.last_results = None



# revision 56
# speedup vs baseline: 2.0025x; 2.0025x over previous
"""Trainium2 Bass kernel for nn_Local2FWLRefine (gnn message passing).

Strategy
--------
The reference computes, per wedge w = (edge i->k, edge k->j) with (i,j) in E2:
    z[w]   = rho_in[w] @ w1 + b1          (rho_in 865 wide)
    msg[w] = silu(z[w]) @ w2 + b2
    M      = segment_sum(msg, eij)        ([E2, 128])
    out    = t_e2 + sigmoid(M@wgw+bgw) * tanh(t_e2@wgt+bgt)

The 865-wide matmul decomposes into per-edge projections:
    z[w] = Q1[eik[w]] + Q2[ekj[w]] + Q3[eij[w]] + c[w] * w1[864]
where Q1/Q2 are per-e1-edge tables and Q3 is per-e2-edge (b1 folded in),
and segment_sum(silu(z) @ w2) = segment_sum(silu(z)) @ w2.

Sharding: wedges sorted by eij; E2 split into 512-edge groups, groups
assigned contiguously to the 8 cores, so each core owns a disjoint slice
of the output rows (no all-reduce).  Phase 1 computes the Q tables on
device from host-staged (compacted, pre-transposed) feature blocks;
phase 2 gathers 3x128 floats per wedge with dma_gather, runs the MLP,
and accumulates the segment sum as silu_z^T @ S (S a 0/1 matrix built
with iota + is_equal) into PSUM, then applies the gated tail in
transposed orientation.
"""

import math
import os
import sys

sys.path.insert(0, "/opt/trn_rl_repo")

import ml_dtypes
import numpy as np

import concourse.bass as bass
import concourse.mybir as mybir
import concourse.tile as tile
from concourse import bacc
from concourse.bass_utils import run_bass_kernel_spmd
from concourse.tile import add_dep_helper
from concourse.masks import make_identity

P = 128
HID = 128
NRBF = 32
GRP = 512           # e2 edges per group (one PSUM bank of fp32)
NCORES = 8
F32 = mybir.dt.float32
F32R = mybir.dt.float32r
I16 = mybir.dt.int16


# ---------------------------------------------------------------- host index math
def _wedge_indices(edge_index1, edge_index2, num_nodes):
    src1 = np.asarray(edge_index1[0])
    dst1 = np.asarray(edge_index1[1])
    src2 = np.asarray(edge_index2[0])
    dst2 = np.asarray(edge_index2[1])
    nz = src1 != dst1
    s, d = src1[nz], dst1[nz]
    eid = np.nonzero(nz)[0]
    out_deg = np.bincount(s, minlength=num_nodes)
    out_order = np.argsort(s, kind="stable")
    out_ptr = np.concatenate([np.zeros(1, np.int64), np.cumsum(out_deg)])
    reps = out_deg[d]
    total = int(reps.sum())
    if total == 0:
        z = np.zeros(0, np.int64)
        return z, z, z, z, z, z
    starts = np.cumsum(reps) - reps
    local = np.arange(total) - np.repeat(starts, reps)
    kj_f = out_order[np.repeat(out_ptr[d], reps) + local]
    i = np.repeat(s, reps)
    k = np.repeat(d, reps)
    eik = np.repeat(eid, reps)
    j = d[kj_f]
    ekj = eid[kj_f]
    m = i != j
    i, k, j, eik, ekj = i[m], k[m], j[m], eik[m], ekj[m]
    e2_keys = src2.astype(np.int64) * num_nodes + dst2
    pk = i.astype(np.int64) * num_nodes + j
    pos = np.searchsorted(e2_keys, pk)
    posc = np.minimum(pos, e2_keys.size - 1)
    valid = (pos < e2_keys.size) & (e2_keys[posc] == pk)
    return i[valid], k[valid], j[valid], eik[valid], ekj[valid], posc[valid]


def _wrap16(arr):
    """int16 index array -> [128, n/16] layout dma_gather expects
    (index i at partition i%16, col i//16; replicated to all 8 Q7 cores)."""
    a = arr.astype(np.int16).reshape(-1, 16).T
    return np.ascontiguousarray(np.tile(a, (8, 1)))


def host_prep(t_e2, h, edge_index1, edge_index2, e1_to_e2, rbf_e1, rbf_e2,
              sph_e1, num_nodes, w1, b1, w2, b2, wgw, bgw, wgt, bgt):
    E2 = t_e2.shape[0]
    N = int(num_nodes)
    E1 = rbf_e1.shape[0]
    src1 = np.asarray(edge_index1[0]).astype(np.int64)
    dst1 = np.asarray(edge_index1[1]).astype(np.int64)
    e1e2 = np.asarray(e1_to_e2).astype(np.int64)

    i_, k_, j_, eik, ekj, eij = _wedge_indices(edge_index1, edge_index2, N)
    W0 = eik.size
    if W0 == 0:
        return None  # caller returns t_e2 unchanged

    c_w = (np.asarray(sph_e1)[eik, 1] * np.asarray(sph_e1)[ekj, 1]).astype(np.float32)
    order = np.argsort(eij, kind="stable")
    eik, ekj, eij, c_w = eik[order], ekj[order], eij[order], c_w[order]

    NGT = math.ceil(E2 / GRP)
    NG = math.ceil(NGT / NCORES)
    NGE = NG * GRP

    gix = eij // GRP                      # global group slot of each wedge (sorted)
    nslots = NCORES * NG
    counts = np.bincount(gix, minlength=nslots)
    SUBG = max(1, int(math.ceil(counts.max() / P)))
    GW = SUBG * P
    WP = NG * GW
    NBLK = WP // P

    # group slot boundaries in the sorted wedge arrays
    bnd = np.searchsorted(gix, np.arange(nslots + 1))

    cnt_full = np.bincount(eij, minlength=E2).astype(np.float32)

    E1 = src1.size
    cores = []
    U12s = []
    rots = []
    for c in range(NCORES):
        base_e = c * NGE
        w_lo, w_hi = bnd[c * NG], bnd[(c + 1) * NG]
        ceik, cekj, ceij, ccw = (eik[w_lo:w_hi], ekj[w_lo:w_hi],
                                 eij[w_lo:w_hi], c_w[w_lo:w_hi])
        U12 = np.unique(np.concatenate([ceik, cekj])) if ceik.size else \
            np.zeros(1, np.int64)
        # order the per-core edge table by ring-relative id so a core whose
        # wedges straddle the wrap still sees contiguous table positions
        rot = int(ceik.min()) if ceik.size else 0
        U12 = U12[np.argsort((U12 - rot) % E1, kind="stable")]
        rots.append(rot)
        U12s.append(U12)
        cores.append((base_e, w_lo, w_hi, ceik, cekj, ceij, ccw, U12))

    # ---- per-group table segments ----
    # Each core's q1/q2 table is laid out in NG segments with SHARED bases
    # B_g: segment g holds the core's U12 rows [c_g, hi_g] (c_g = first row
    # any group-g wedge reads as Q1; hi_g covers the overlap into the next
    # segment), so per-block Q1 row windows line up across cores.  Rows used
    # by several groups are duplicated into each segment.
    seg_lo = np.zeros((NCORES, NG), np.int64)   # c_g per core
    seg_hi = np.zeros((NCORES, NG), np.int64)   # hi_g per core (inclusive)

    def u12_pos(c, ids):
        """positions of edge ids in the (ring-rotated) per-core table"""
        U12, rot = U12s[c], rots[c]
        keys = (U12 - rot) % E1
        return np.searchsorted(keys, (ids - rot) % E1)

    for c, (base_e, w_lo, w_hi, ceik, cekj, ceij, ccw, U12) in enumerate(cores):
        p1 = u12_pos(c, ceik)
        p2 = u12_pos(c, cekj)
        for g in range(NG):
            lo = bnd[c * NG + g] - w_lo
            hi = bnd[c * NG + g + 1] - w_lo
            if hi > lo:
                seg_lo[c, g] = int(p1[lo:hi].min())
                seg_hi[c, g] = int(max(p1[lo:hi].max(), p2[lo:hi].max()))
            else:
                seg_lo[c, g] = seg_lo[c, g - 1] if g else 0
                seg_hi[c, g] = seg_lo[c, g]
    # segment g must cover all rows in [c_g, c_{g+1}) plus the Q1/Q2 overlap
    nxt = np.concatenate([seg_lo[:, 1:],
                          np.array([[u.size for u in U12s]]).T], axis=1)
    seg_len = np.maximum(seg_hi + 1, nxt) - seg_lo        # [NCORES, NG]
    L = seg_len.max(axis=0)                               # shared lengths
    B = np.zeros(NG + 1, np.int64)
    for g in range(NG):
        B[g + 1] = B[g] + ((int(L[g]) + 63) // 64) * 64
    T = max(512, int(math.ceil(B[NG] / (4 * P))) * 4 * P)
    if T >= 32768:
        raise RuntimeError(f"per-core Q table too large for int16 gather: {T}")
    NB1 = T // P
    NB2 = NGE // P

    def u12_to_slot(c):
        """[NG, U12.size] -> slot id (or -1) per (segment, row)."""
        n = U12s[c].size
        slot_of = np.full((NG, n), -1, np.int64)
        for g in range(NG):
            lo, ln = seg_lo[c, g], seg_len[c, g]
            slot_of[g, lo:lo + ln] = B[g] + np.arange(ln)
        return slot_of

    # padded per-(core,group,subtile) el / q1-row values to derive shared
    # window bases
    el_pad = np.full((NCORES, NG, SUBG, P), np.nan, np.float32)
    r1_pad = np.full((NCORES, NG, SUBG, P), np.nan, np.float32)
    percore = []
    for c, (base_e, w_lo, w_hi, ceik, cekj, ceij, ccw, U12) in enumerate(cores):
        q2i = np.zeros(WP, np.int16)
        cwp = np.zeros(WP, np.float32)
        elg = np.full(WP, np.nan, np.float32)   # el within group [0, GRP)
        r1g = np.full(WP, np.nan, np.float32)   # q1 table slot
        p1 = u12_pos(c, ceik)
        p2 = u12_pos(c, cekj)
        slot_of = u12_to_slot(c)
        loc = ceij - base_e
        hi12c = np.zeros(NG, np.int64)
        for g in range(NG):
            lo = bnd[c * NG + g] - w_lo
            hi = bnd[c * NG + g + 1] - w_lo
            n = hi - lo
            dst = g * GW
            s1 = slot_of[g, p1[lo:hi]]
            s2 = slot_of[g, p2[lo:hi]]
            assert n == 0 or (s1.min() >= 0 and s2.min() >= 0)
            q2i[dst:dst + n] = s2
            cwp[dst:dst + n] = ccw[lo:hi]
            elg[dst:dst + n] = (loc[lo:hi] - g * GRP).astype(np.float32)
            r1g[dst:dst + n] = s1.astype(np.float32)
            hi12c[g] = s2.max(initial=0)
        el_pad[c] = elg.reshape(NG, SUBG, P)
        r1_pad[c] = r1g.reshape(NG, SUBG, P)
        percore.append((q2i, cwp, hi12c))

    # shared (across cores) per-(g,s) window base; WS = max span, mult of 32
    with np.errstate(invalid="ignore"):
        mn = np.nanmin(el_pad, axis=(0, 3))     # [NG, SUBG]
        mx = np.nanmax(el_pad, axis=(0, 3))
    valid_any = ~np.isnan(el_pad).all(axis=(0, 3))        # [NG, SUBG]
    mn = np.where(np.isnan(mn), 0.0, mn)
    mx = np.where(np.isnan(mx), 0.0, mx)
    span = (mx - mn + 1).max()
    WS = min(GRP, int(math.ceil(span / 32)) * 32)
    base_gs = np.minimum(mn, GRP - WS).astype(np.int32)   # [NG, SUBG]

    # Q3 64-aligned windows per block: els of block (g,s) fit in
    # [wb, wb+128) with wb = 64*floor(mn/64) (cross-core span <= WS <= 64,
    # enforced below).  Window A = [wb, wb+64), window B = [wb+64, wb+128)
    # (only when some el >= wb+64).  The one-hot rows live at el%128 so the
    # lhsT slice's base partition always matches the rhs table slice's.
    if WS > 64:
        raise RuntimeError(f"q3 window scheme needs span <= 64, got WS={WS}")
    wb_gs = (64 * (mn.astype(np.int64) // 64)).astype(np.int32)   # [NG, SUBG]
    useB_gs = (mx >= wb_gs + 64) & valid_any
    useA_gs = valid_any

    # same 64-aligned window scheme for the Q1 table rows
    with np.errstate(invalid="ignore"):
        mn1 = np.nanmin(r1_pad, axis=(0, 3))
        mx1 = np.nanmax(r1_pad, axis=(0, 3))
    mn1 = np.where(np.isnan(mn1), 0.0, mn1)
    mx1 = np.where(np.isnan(mx1), 0.0, mx1)
    if (mx1 - mn1 + 1).max() > 64:
        raise RuntimeError(
            f"q1 window scheme needs span <= 64, got {(mx1 - mn1 + 1).max()}")
    wb1_gs = (64 * (mn1.astype(np.int64) // 64)).astype(np.int32)
    useB1_gs = (mx1 >= wb1_gs + 64) & valid_any
    useA1_gs = valid_any

    # per-group q12 block-batch milestone (in 4*P-row units, shared over cores)
    hi12 = np.max(np.stack([pc[2] for pc in percore]), axis=0)
    need_b4 = tuple(int(x) for x in np.minimum(hi12 // (4 * P) + 1, NB1 // 4))

    meta = dict(NG=NG, SUBG=SUBG, T=T, NB1=NB1, NB2=NB2, NGE=NGE, WP=WP,
                NBLK=NBLK, WS=WS, bases=tuple(map(int, base_gs.reshape(-1))),
                need_b4=need_b4,
                q3wb=tuple(map(int, wb_gs.reshape(-1))),
                q3useA=tuple(map(bool, useA_gs.reshape(-1))),
                q3useB=tuple(map(bool, useB_gs.reshape(-1))),
                q1wb=tuple(map(int, wb1_gs.reshape(-1))),
                q1useA=tuple(map(bool, useA1_gs.reshape(-1))),
                q1useB=tuple(map(bool, useB1_gs.reshape(-1))))

    # ---- weights (shared) ----
    w1 = np.asarray(w1, np.float32)
    wcat = np.zeros((4 * P, 2 * P), np.float32)
    wcat[0:128, 0:128] = w1[0:128]          # t_e2[e1e2[e]]  -> Q1
    wcat[0:128, 128:256] = w1[128:256]      # t_e2[e1e2[e]]  -> Q2
    wcat[128:160, 0:128] = w1[768:800]      # rbf_e1[e]      -> Q1
    wcat[128:160, 128:256] = w1[800:832]    # rbf_e1[e]      -> Q2
    wcat[160:288, 0:128] = w1[384:512]      # h[src1[e]]     -> Q1
    wcat[288:416, 0:128] = w1[512:640]      # h[dst1[e]]     -> Q1 (h_k)
    wcat[288:416, 128:256] = w1[640:768]    # h[dst1[e]]     -> Q2 (h_j)
    wcat[416, 0:128] = np.asarray(b1, np.float32)   # b1 via const column
    # gate sigmoid via tanh identity: sigmoid(x) = 0.5*(1 + tanh(x/2)); the
    # 1/2 is folded into wgw/bgw, and M = U@w2 + cnt x b2 is never
    # materialized: M@(wgw/2) = U@(w2@wgw/2) + cnt x (b2@wgw/2).
    wgwh = np.asarray(wgw, np.float32) * 0.5
    bgwh = np.asarray(bgw, np.float32) * 0.5
    w2w = (np.asarray(w2, np.float32) @ wgwh).astype(np.float32)
    b2w = (np.asarray(b2, np.float32) @ wgwh).astype(np.float32)
    shared = {
        "wcat": np.ascontiguousarray(wcat).astype(ml_dtypes.bfloat16),
        "w1c": np.ascontiguousarray(w1[256:384]).astype(ml_dtypes.bfloat16),
        "w1f": np.ascontiguousarray(w1[832:864]).astype(ml_dtypes.bfloat16),
        "w2w": w2w,
        "b2w": b2w[None, :],
        "wgt": np.asarray(wgt, np.float32).astype(ml_dtypes.bfloat16),
        "bgwc": np.ascontiguousarray(bgwh[:, None]),
        "bgtc": np.ascontiguousarray(np.asarray(bgt, np.float32)[:, None]),
        "w1lr": np.ascontiguousarray(w1[864:865, :]).astype(ml_dtypes.bfloat16),
    }

    t_e2 = np.asarray(t_e2, np.float32)
    h = np.asarray(h, np.float32)
    rbf_e1 = np.asarray(rbf_e1, np.float32)
    rbf_e2 = np.asarray(rbf_e2, np.float32)

    el_rel = el_pad.reshape(NCORES, NG, SUBG, P) - base_gs[None, :, :, None]
    el_rel = np.where(np.isnan(el_rel), -5.0, el_rel).astype(np.float32)

    in_maps = []
    for c, (base_e, w_lo, w_hi, ceik, cekj, ceij, ccw, U12) in enumerate(cores):
        q2i, cwp, _ = percore[c]
        # slot t -> U12 row (duplicated across segment overlaps)
        slotrow = np.full(T, -1, np.int64)
        for g in range(NG):
            ln = int(seg_len[c, g])
            slotrow[B[g]:B[g] + ln] = seg_lo[c, g] + np.arange(ln)
        mask = slotrow >= 0
        U12e = U12[np.clip(slotrow, 0, U12.size - 1)]
        gtab = np.zeros((T, 4 * P), np.float32)
        gtab[:, 416] = 1.0          # constant column carrying b1
        gtab[mask, 0:128] = t_e2[e1e2[U12e[mask]]]
        gtab[mask, 128:160] = rbf_e1[U12e[mask]]
        gtab[mask, 160:288] = h[src1[U12e[mask]]]
        gtab[mask, 288:416] = h[dst1[U12e[mask]]]
        gtabT = np.ascontiguousarray(
            gtab.reshape(NB1, P, 4 * P).transpose(0, 2, 1)).astype(
                ml_dtypes.bfloat16)

        hi_e = min(base_e + NGE, E2)
        nreal = hi_e - base_e
        tslab = np.zeros((NGE, P), np.float32)
        rbf2s = np.zeros((NGE, NRBF), np.float32)
        cntc = np.zeros(NGE, np.float32)
        if nreal > 0:
            tslab[:nreal] = t_e2[base_e:hi_e]
            rbf2s[:nreal] = rbf_e2[base_e:hi_e]
            cntc[:nreal] = cnt_full[base_e:hi_e]

        # one-hot expansion matrices, interleaved per subtile:
        #   sel13[g, el%128, s, 0, p] = 1   (Q3: group-local edge index el)
        #   sel13[g, row%128, s, 1, p] = 1  (Q1: q1-table row)
        sel13 = np.zeros((NG, P, SUBG, 2, P), ml_dtypes.bfloat16)
        elc = el_pad[c]                     # [NG, SUBG, P], el within group
        gg, ss, pp = np.nonzero(~np.isnan(elc))
        rr = elc[gg, ss, pp].astype(np.int64) % P
        sel13[gg, rr, ss, 0, pp] = 1.0
        r1c = r1_pad[c]
        gg, ss, pp = np.nonzero(~np.isnan(r1c))
        rr = r1c[gg, ss, pp].astype(np.int64) % P
        sel13[gg, rr, ss, 1, pp] = 1.0

        in_maps.append({
            "gtabT": gtabT,
            "t16": np.ascontiguousarray(tslab.T).astype(ml_dtypes.bfloat16),
            "rbf16": np.ascontiguousarray(rbf2s.T).astype(ml_dtypes.bfloat16),
            "cnt": np.ascontiguousarray(cntc[None, :]),
            "q2i": _wrap16(q2i),
            "sel13": np.ascontiguousarray(sel13).reshape(NG, P, SUBG, 2 * P),
            "cwt": np.ascontiguousarray(cwp[None, :]).astype(
                ml_dtypes.bfloat16),
            "elw": np.ascontiguousarray(
                el_rel[c].reshape(NBLK, P).T),
            **shared,
        })
    return in_maps, meta, E2


# ---------------------------------------------------------------- device program
def build_program(meta, use_silu=True):
    NG, SUBG, T = meta["NG"], meta["SUBG"], meta["T"]
    NB1, NB2, NGE = meta["NB1"], meta["NB2"], meta["NGE"]
    WP, NBLK, WS = meta["WP"], meta["NBLK"], meta["WS"]
    bases = meta["bases"]
    GW = SUBG * P
    AF = mybir.ActivationFunctionType

    nc = bacc.Bacc("TRN2", target_bir_lowering=False, debug=False,
                   enable_asserts=False, num_devices=NCORES)

    def din(name, shape, dt=F32):
        return nc.dram_tensor(name, shape, dt, kind="ExternalInput").ap()

    BF16 = mybir.dt.bfloat16
    gtabT = din("gtabT", [NB1, 4 * P, P], BF16)
    t16d = din("t16", [P, NGE], BF16)
    rbf16d = din("rbf16", [NRBF, NGE], BF16)
    cnt = din("cnt", [1, NGE], F32R)
    q2i = din("q2i", [P, WP // 16], I16)
    sel13 = din("sel13", [NG, P, SUBG, 2 * P], BF16)
    cwt = din("cwt", [1, WP], BF16)
    elw = din("elw", [P, NBLK])
    wcat = din("wcat", [4 * P, 2 * P], BF16)
    w1c = din("w1c", [P, P], BF16)
    w1f = din("w1f", [NRBF, P], BF16)
    w2w = din("w2w", [P, P], F32R)
    b2w = din("b2w", [1, P], F32R)
    wgt = din("wgt", [P, P], BF16)
    bgwc = din("bgwc", [P, 1])
    bgtc = din("bgtc", [P, 1])
    w1lr = din("w1lr", [1, P], BF16)
    outT = nc.dram_tensor("outT", [P, NGE], BF16, kind="ExternalOutput").ap()

    with tile.TileContext(nc) as tc:
        with (
            tc.tile_pool(name="const", bufs=1) as cpool,
            tc.tile_pool(name="dram", bufs=1, space="DRAM") as dpool,
            tc.tile_pool(name="p1in", bufs=3) as p1in,
            tc.tile_pool(name="p1out", bufs=2) as p1out,
            tc.tile_pool(name="gath", bufs=3) as gath,
            tc.tile_pool(name="zbuf", bufs=3) as zbuf,
            tc.tile_pool(name="sbuf", bufs=3) as spool,
            tc.tile_pool(name="tail", bufs=2) as tpool,
            tc.tile_pool(name="ps1", bufs=2, space="PSUM") as ps1,
            tc.tile_pool(name="psu", bufs=2, space="PSUM") as psu,
            tc.tile_pool(name="psz", bufs=2, space="PSUM") as pszp,
            tc.tile_pool(name="pstail", bufs=2, space="PSUM") as pstail,
        ):
            # ---------------- constants ----------------
            wcat_sb = cpool.tile([P, 4, 2 * P], mybir.dt.bfloat16)
            nc.sync.dma_start(wcat_sb[:],
                              wcat.rearrange("(c p) f -> p c f", p=P))
            w1c_sb = cpool.tile([P, P], mybir.dt.bfloat16)
            nc.sync.dma_start(w1c_sb[:], w1c[:, :])
            w1f_sb = cpool.tile([NRBF, P], mybir.dt.bfloat16)
            nc.sync.dma_start(w1f_sb[:], w1f[:, :])
            w2w_sb = cpool.tile([P, P], F32R)
            nc.sync.dma_start(w2w_sb[:], w2w[:, :])
            b2w_sb = cpool.tile([1, P], F32R)
            nc.sync.dma_start(b2w_sb[:], b2w[:, :])
            wgt_sb = cpool.tile([P, P], mybir.dt.bfloat16)
            nc.sync.dma_start(wgt_sb[:], wgt[:, :])
            t16_sb = cpool.tile([P, NGE], mybir.dt.bfloat16)
            nc.sync.dma_start(t16_sb[:], t16d[:, :])
            bgw_sb = cpool.tile([P, 1], F32)
            nc.sync.dma_start(bgw_sb[:], bgwc[:, :])
            bgt_sb = cpool.tile([P, 1], F32)
            nc.sync.dma_start(bgt_sb[:], bgtc[:, :])
            w1lr_sb = cpool.tile([1, P], mybir.dt.bfloat16)
            nc.sync.dma_start(w1lr_sb[:], w1lr[:, :])
            cnt_sb = cpool.tile([1, NGE], F32R)
            nc.sync.dma_start(cnt_sb[:], cnt[:, :])

            elw_sb = cpool.tile([P, NBLK], F32)
            nc.sync.dma_start(elw_sb[:], elw[:, :])
            q2i_sb = cpool.tile([P, WP // 16], I16)
            nc.sync.dma_start(q2i_sb[:], q2i[:, :])
            zero_f = cpool.tile([1, GRP], F32)
            nc.gpsimd.memset(zero_f[:], 0.0)
            zero_sb = cpool.tile([1, GRP], F32R)
            nc.vector.tensor_copy(zero_sb[:], zero_f[:])
            ident_sb = cpool.tile([P, P], mybir.dt.bfloat16)
            make_identity(nc, ident_sb[:])
            iota_sb = cpool.tile([P, WS], F32)
            nc.gpsimd.iota(iota_sb[:], pattern=[[1, WS]], base=0,
                           channel_multiplier=0,
                           allow_small_or_imprecise_dtypes=True)

            # DRAM scratch table (q2 only; q1/q3 live in SBUF, row r at
            # partition r%128, free chunk r//128)
            q2t = dpool.tile([T, P], mybir.dt.bfloat16)
            q1sb = cpool.tile([P, NB1, P], mybir.dt.bfloat16)
            q3sb = cpool.tile([P, NB2, P], mybir.dt.bfloat16)

            # ---------------- phase 1: Q tables ----------------
            # q12 and q3 block batches are emitted interleaved, and each
            # batch's DRAM write is kept as a milestone so that phase-2
            # group g only waits for the table prefix it actually reads
            # (dma_gather's DRAM source read is not tracked by Tile's
            # dependency hook, hence the explicit deps).
            q12_ms = [None] * (NB1 // 4)
            q3_ms = [None] * (NB2 // 8)

            def emit_q12(b4i):
                q2c = p1out.tile([P, 4, P], mybir.dt.bfloat16, tag="q2c")
                gt = p1in.tile([P, 4, 4, P], mybir.dt.bfloat16, tag="gt")
                nc.sync.dma_start(
                    gt[:], gtabT[b4i * 4:b4i * 4 + 4]
                    .rearrange("n (c p) f -> p n c f", p=P))
                for half in range(4):
                    pq = ps1.tile([P, 2 * P], F32, tag="pq")
                    for ci in range(4):
                        nc.tensor.matmul(
                            pq[:], lhsT=gt[:, half, ci, :],
                            rhs=wcat_sb[:, ci, :],
                            start=(ci == 0), stop=(ci == 3))
                    nc.vector.tensor_copy(q1sb[:, b4i * 4 + half, :],
                                          pq[:, 0:P])
                    nc.vector.tensor_copy(q2c[:, half, :], pq[:, P:2 * P])
                q12_ms[b4i] = nc.scalar.dma_start(
                    q2t[b4i * 4 * P:(b4i + 1) * 4 * P, :]
                    .rearrange("(c p) f -> p c f", p=P),
                    q2c[:])

            def emit_q3(b8):
                rts8 = p1in.tile([NRBF, 8 * P], mybir.dt.bfloat16, tag="rts")
                nc.sync.dma_start(rts8[:],
                                  rbf16d[:, b8 * 8 * P:(b8 + 1) * 8 * P])
                for qi in range(8):
                    c0 = (b8 * 8 + qi) * P
                    pq3 = ps1.tile([P, P], F32, tag="pq")
                    nc.tensor.matmul(pq3[:], lhsT=t16_sb[:, c0:c0 + P],
                                     rhs=w1c_sb[:], start=True, stop=False)
                    nc.tensor.matmul(pq3[:], lhsT=rts8[:, qi * P:(qi + 1) * P],
                                     rhs=w1f_sb[:], start=False, stop=True)
                    q3_ms[b8] = nc.vector.tensor_copy(
                        q3sb[:, b8 * 8 + qi, :], pq3[:])

            for step in range(max(NB1 // 4, NB2 // 8)):
                if step < NB1 // 4:
                    emit_q12(step)
                if step < NB2 // 8:
                    emit_q3(step)

            # single funnel fence: all q2-table DMA writes before any gather
            # (per-group milestone deps hang real hardware)
            fence_a = cpool.tile([1, 1], F32)
            nc.gpsimd.memset(fence_a[:], 0.0)
            fence_b = cpool.tile([1, 1], F32)
            fence = nc.vector.tensor_copy(fence_b[:], fence_a[:])
            for wi in q12_ms:
                add_dep_helper(fence.ins, wi.ins, sync=True,
                               reason="q2 table before gathers")

            # ---------------- phase 2: wedges + tail ----------------
            for g in range(NG):
                ic0 = g * GW // 16
                ic1 = (g + 1) * GW // 16
                g2 = gath.tile([P, SUBG, P], mybir.dt.bfloat16, tag="g2")
                gi2 = nc.gpsimd.dma_gather(
                    out_ap=g2[:], in_ap=q2t[:, :],
                    idxs_ap=q2i_sb[:, ic0:ic1],
                    num_idxs=GW, num_idxs_reg=GW, elem_size=P,
                    single_packet=False)
                selg = gath.tile([P, SUBG, 2, P], mybir.dt.bfloat16,
                                 tag="selg")
                nc.sync.dma_start(
                    selg[:],
                    sel13[g:g + 1].rearrange("n p s f -> p (n s) f")
                    .rearrange("p s (two f) -> p s two f", two=2))
                add_dep_helper(gi2.ins, fence.ins, sync=True,
                               reason="q2 table before gather")

                cwt_g = spool.tile([1, GW], mybir.dt.bfloat16, tag="cwt")
                nc.sync.dma_start(cwt_g[:], cwt[:, g * GW:(g + 1) * GW])
                pu = psu.tile([P, GRP], F32, tag="pu")
                nc.tensor.matmul(pu[:], lhsT=zero_sb[:, 0:P],
                                 rhs=zero_sb[:, 0:GRP],
                                 start=True, stop=False)

                quads = []
                q0 = 0
                while q0 < SUBG:
                    qw = min(4, SUBG - q0)
                    psz = pszp.tile([P, qw * P], F32, tag="psz")
                    # one PSUM accumulation group per psz tile: the first
                    # (start=True) matmul pending-zeroes the whole bank, the
                    # last rank-1 carries stop.  Matmuls are grouped by
                    # stationary operand to limit weight reloads.
                    for h0 in range(0, qw, 2):
                        hw_ = min(2, qw - h0)
                        dst = psz[:, h0 * P:(h0 + hw_) * P]
                        nc.tensor.matmul(dst, lhsT=ident_sb[:],
                                         rhs=g2[:, q0 + h0:q0 + h0 + hw_, :],
                                         start=(h0 == 0), stop=False)
                    for bi in range(qw):
                        s = q0 + bi
                        blk = g * SUBG + s
                        dst = psz[:, bi * P:(bi + 1) * P]
                        # one-hot window matmuls; when A and B sit in the
                        # same table rank (base partition 0) they merge into
                        # a single K=128 matmul
                        def sel_windows(base, useA, useB, table, sl):
                            if not useA:
                                return
                            p0 = base % P
                            rk = base // P
                            if useB and p0 == 0:
                                nc.tensor.matmul(
                                    dst, lhsT=selg[:, s, sl, :],
                                    rhs=table[:, rk, :],
                                    start=False, stop=False)
                                return
                            nc.tensor.matmul(
                                dst, lhsT=selg[p0:p0 + 64, s, sl, :],
                                rhs=table[p0:p0 + 64, rk, :],
                                start=False, stop=False)
                            if useB:
                                b0 = base + 64
                                q0_ = b0 % P
                                nc.tensor.matmul(
                                    dst, lhsT=selg[q0_:q0_ + 64, s, sl, :],
                                    rhs=table[q0_:q0_ + 64, b0 // P, :],
                                    start=False, stop=False)
                        sel_windows(g * GRP + meta["q3wb"][blk],
                                    meta["q3useA"][blk], meta["q3useB"][blk],
                                    q3sb, 0)
                        sel_windows(meta["q1wb"][blk],
                                    meta["q1useA"][blk], meta["q1useB"][blk],
                                    q1sb, 1)
                    for bi in range(qw):
                        sblk = q0 + bi
                        nc.tensor.matmul(
                            psz[:, bi * P:(bi + 1) * P],
                            lhsT=cwt_g[:, sblk * P:(sblk + 1) * P],
                            rhs=w1lr_sb[:],
                            start=False, stop=(bi == qw - 1))
                    silu = zbuf.tile([P, qw, P], mybir.dt.bfloat16, tag="silu")
                    if use_silu:
                        nc.scalar.activation(
                            silu[:].rearrange("p a b -> p (a b)"), psz[:],
                            AF.Silu)
                    else:
                        sig = zbuf.tile([P, qw, P], F32, tag="sig")
                        nc.scalar.activation(
                            sig[:].rearrange("p a b -> p (a b)"), psz[:],
                            AF.Sigmoid)
                        nc.vector.tensor_tensor(
                            out=silu[:].rearrange("p a b -> p (a b)"),
                            in0=sig[:].rearrange("p a b -> p (a b)"),
                            in1=psz[:], op=mybir.AluOpType.mult)
                    quads.append((q0, qw, silu))
                    q0 += qw

                for s in range(SUBG):
                    blk = g * SUBG + s
                    base = bases[g * SUBG + s]
                    ssb = spool.tile([P, WS], mybir.dt.bfloat16, tag="ssb")
                    nc.vector.tensor_scalar(
                        out=ssb[:], in0=iota_sb[:],
                        scalar1=elw_sb[:, blk:blk + 1], scalar2=None,
                        op0=mybir.AluOpType.is_equal)
                    qidx = s // 4
                    sq0, sqw, silu_q = quads[qidx]
                    nc.tensor.matmul(
                        pu[:, base:base + WS],
                        lhsT=silu_q[:, s - sq0, :], rhs=ssb[:],
                        start=False, stop=(s == SUBG - 1))

                # tail for this group's 512 edges:
                #   th = tanh(U@W2W + cnt x B2W + bgw/2)    (= 2*sigmoid-1)
                #   T  = tanh(t@wgt + bgt)
                #   out = t + 0.5*(1+th)*T
                u_sb = tpool.tile([P, GRP], F32R, tag="u")
                nc.vector.tensor_copy(u_sb[:], pu[:])
                pg = pstail.tile([P, GRP], F32, tag="ptail")
                for h0 in (0, 2 * P):
                    nc.tensor.matmul(pg[:, h0:h0 + 2 * P], lhsT=w2w_sb[:],
                                     rhs=u_sb[:, h0:h0 + 2 * P],
                                     start=True, stop=False)
                    nc.tensor.matmul(pg[:, h0:h0 + 2 * P], lhsT=b2w_sb[:],
                                     rhs=cnt_sb[:, g * GRP + h0:
                                                g * GRP + h0 + 2 * P],
                                     start=False, stop=True)
                th = tpool.tile([P, GRP], F32, tag="gate")
                nc.scalar.activation(th[:], pg[:], AF.Tanh, bias=bgw_sb[:])

                pt = pstail.tile([P, GRP], F32, tag="ptail")
                for h0 in (0, 2 * P):
                    nc.tensor.matmul(
                        pt[:, h0:h0 + 2 * P], lhsT=wgt_sb[:],
                        rhs=t16_sb[:, g * GRP + h0:g * GRP + h0 + 2 * P],
                        start=True, stop=True)
                tact = tpool.tile([P, GRP], F32, tag="tact")
                nc.scalar.activation(tact[:], pt[:], AF.Tanh, bias=bgt_sb[:])

                t32g = tpool.tile([P, GRP], F32, tag="t32")
                nc.scalar.activation(t32g[:],
                                     t16_sb[:, g * GRP:(g + 1) * GRP],
                                     AF.Copy)
                o_sb = tpool.tile([P, GRP], F32, tag="o")
                nc.vector.tensor_tensor(out=o_sb[:], in0=th[:], in1=tact[:],
                                        op=mybir.AluOpType.mult)
                nc.gpsimd.tensor_add(o_sb[:], o_sb[:], tact[:])
                nc.vector.tensor_scalar(
                    out=o_sb[:], in0=o_sb[:], scalar1=0.5, scalar2=None,
                    op0=mybir.AluOpType.mult)
                o16 = tpool.tile([P, GRP], mybir.dt.bfloat16, tag="o16")
                nc.vector.tensor_add(o16[:], o_sb[:], t32g[:])
                nc.scalar.dma_start(outT[:, g * GRP:(g + 1) * GRP], o16[:])

    nc.compile()
    return nc


_CACHE = {}


def _get_program(meta, use_silu=True):
    key = (tuple(sorted((k, v) for k, v in meta.items() if k != "bases")),
           meta["bases"], use_silu)
    if key not in _CACHE:
        _CACHE[key] = build_program(meta, use_silu=use_silu)
    return _CACHE[key]


def kernel(**inputs):
    np_inputs = {k: np.asarray(v) for k, v in inputs.items()}
    t_e2 = np.asarray(np_inputs["t_e2"], np.float32)
    prep = host_prep(
        t_e2, np_inputs["h"], np_inputs["edge_index1"],
        np_inputs["edge_index2"], np_inputs["e1_to_e2"], np_inputs["rbf_e1"],
        np_inputs["rbf_e2"], np_inputs["sph_e1"], np_inputs["num_nodes"],
        np_inputs["w1"], np_inputs["b1"], np_inputs["w2"], np_inputs["b2"],
        np_inputs["wgw"], np_inputs["bgw"], np_inputs["wgt"], np_inputs["bgt"])
    if prep is None:
        return t_e2
    in_maps, meta, E2 = prep
    use_silu = os.environ.get("KERNEL_NO_SILU", "0") != "1"
    nc = _get_program(meta, use_silu=use_silu)
    trace = os.environ.get("KERNEL_TRACE", "0") == "1"
    res = run_bass_kernel_spmd(nc, in_maps, core_ids=list(range(NCORES)),
                               trace=trace)
    kernel.last_results = res
    NGE = meta["NGE"]
    out = np.empty((E2, HID), np.float32)
    for c in range(NCORES):
        base = c * NGE
        hi = min(base + NGE, E2)
        if hi <= base:
            break
        out[base:hi, :] = res.results[c]["outT"][:, :hi - base].T
    return out


kernel.last_results = None

